# revision 10
# baseline (speedup 1.0000x reference)
"""Trainium2 Bass kernel for nn_ConvDeIndexer.

Math (verified bit-exact vs the jax reference):
  X[b, 64g+c, w, h] = scatter of 8 candidates per (b,g,w,h):
      for t in 0..7 (pos k=0..3 then neg k=0..3): X[ind_t] = val_t  (last wins)
  y[b, 32g+co, 4w+i, 4h+j] = sum_c X[b, 64g+c, w, h] * weight[64g+c, co, i, j]
  (stride == ksize == 4 -> conv_transpose windows never overlap)

Sharding: 32 (b, g) pairs over 8 cores -> core c handles b = c//2 and
groups 4*(c%2) .. 4*(c%2)+3.  Each core writes its own [128, 256, 256]
slice of y (128 output channels).

Per-core pipeline (all on device):
  1. DMA ind/val slices into SBUF, partition p = s//32 (s = w*64 + h),
     free = (t, sigma), sigma = s%32.
  2. DVE: resolve scatter collisions (a later candidate with an equal
     index kills the earlier one -> negative scatter index), convert
     values to fp16.
  3. GPSIMD local_scatter builds xT[s-part, (g2, sigma32, c64)] fp16
     (zeroed dense transposed-X, two groups stacked in the free dim).
  4. TensorE transposes [128, (g2,c64)] blocks (one per sigma) ->
     x_pair[(g2,c64), (sigma, p)] - a K=128 two-group stack in PSUM,
     copied to SBUF.
  5. fp16 matmuls with block-diagonal two-group weights, col-tiled so
     pair 0 -> psum partitions 0-63, pair 1 -> 64-127: psum partition
     layout is exactly (g4, co32) = the output channel.
  6. DVE/ScalarE copies rearrange psum into DRAM-contiguous staging
     buffers; fully-contiguous 4MB DMAs write y.
"""

import numpy as np
from contextlib import ExitStack

N_CORES = 8
P = 128

_CACHE = {}


def _build():
    if "nc" in _CACHE:
        return _CACHE["nc"]
    import concourse.tile as tile
    from concourse import bacc, mybir

    f32 = mybir.dt.float32
    i32 = mybir.dt.int32
    u16 = mybir.dt.uint16

    nc = bacc.Bacc("TRN2", target_bir_lowering=False, debug=False,
                   num_devices=N_CORES)

    ind_pos = nc.dram_tensor("ind_pos", (4, 4, 64, 64), i32, kind="ExternalInput")
    ind_neg = nc.dram_tensor("ind_neg", (4, 4, 64, 64), i32, kind="ExternalInput")
    val_pos = nc.dram_tensor("val_pos", (4, 4, 64, 64), f32, kind="ExternalInput")
    val_neg = nc.dram_tensor("val_neg", (4, 4, 64, 64), f32, kind="ExternalInput")
    weight = nc.dram_tensor("weight", (256, 32, 4, 4), f32, kind="ExternalInput")
    y = nc.dram_tensor("y", (128, 256, 256), f32, kind="ExternalOutput")

    with tile.TileContext(nc) as tc:
        with ExitStack() as ctx:
            _body(nc, tc, ctx, mybir,
                  ind_pos, ind_neg, val_pos, val_neg, weight, y)

    nc.compile()
    _CACHE["nc"] = nc
    return nc


def _body(nc, tc, ctx, mybir, ind_pos, ind_neg, val_pos, val_neg, weight, y):
    from concourse.masks import make_identity
    f32 = mybir.dt.float32
    f16 = mybir.dt.float16
    i32 = mybir.dt.int32
    i16 = mybir.dt.int16
    Alu = mybir.AluOpType

    const_pool = ctx.enter_context(tc.tile_pool(name="const", bufs=1))
    w_pool = ctx.enter_context(tc.tile_pool(name="wpool", bufs=2))
    inp_pool = ctx.enter_context(tc.tile_pool(name="inp", bufs=2))
    prep_pool = ctx.enter_context(tc.tile_pool(name="prep", bufs=2))
    xt_pool = ctx.enter_context(tc.tile_pool(name="xt", bufs=2))
    xsb_pool = ctx.enter_context(tc.tile_pool(name="xsb", bufs=2))
    stage_pool = ctx.enter_context(tc.tile_pool(name="stage", bufs=3))
    psx_pool = ctx.enter_context(tc.tile_pool(name="psx", bufs=2, space="PSUM"))
    psy_pool = ctx.enter_context(tc.tile_pool(name="psy", bufs=4, space="PSUM"))

    # ---- constants ----
    ident = const_pool.tile([P, P], f16)
    make_identity(nc, ident[:])
    # iota_c[h][p, (t8, q8, s4)] = 128 * s + 64 * h   (s = sigma % 4)
    iota_cs = []
    for h in range(2):
        iota_c = const_pool.tile([P, 256], i32, name=f"iota_c{h}")
        nc.gpsimd.iota(iota_c[:], pattern=[[0, 8], [0, 8], [128, 4]],
                       base=64 * h, channel_multiplier=0)
        iota_cs.append(iota_c)

    # ---- weights: block-diagonal two-group stacks, fp16 ----
    # wpair[pr]: [K=128=(gA c64, gB c64), (m64=(gA co32, gB co32), i4, j4)]
    w_tiles = []
    for pr in range(2):
        w_raw = inp_pool.tile([P, 512], f32, name="w_raw", tag="w_raw")
        for h in range(2):
            g = 2 * pr + h
            nc.sync.dma_start(
                w_raw[64 * h:64 * h + 64, :],
                weight.ap()[64 * g:64 * g + 64].rearrange("c co i j -> c (co i j)"))
        wp = w_pool.tile([P, 1024], f16, name=f"wp{pr}", tag="wp")
        nc.vector.memset(wp[:], 0)
        wv = wp[:].rearrange("c (m i j) -> c m i j", m=64, i=4)
        nc.vector.tensor_copy(
            wv[0:64, 0:32],
            w_raw[0:64, :].rearrange("c (co i j) -> c co i j", co=32, i=4))
        nc.vector.tensor_copy(
            wv[64:128, 32:64],
            w_raw[64:128, :].rearrange("c (co i j) -> c co i j", co=32, i=4))
        w_tiles.append(wp)

    # ---- per-pair input prep, scatter, transpose ----
    x_tiles = []
    for pr in range(2):
        # xT: [128, (sig32, g2, c64)] fp16
        xT = xt_pool.tile([P, 4096], f16, name="xT", tag="xT")
        # scatter data/idx: [128, (q8, g2, t8, s4)]
        idxs_p = prep_pool.tile([P, 512], i16, name="idxs_p")
        data_p = prep_pool.tile([P, 512], f16, name="data_p")
        for h in range(2):
            g = 2 * pr + h
            # load indices [128, (t8, sig32)]
            ind_raw = inp_pool.tile([P, 256], i32, name="ind_raw")
            ind_v = ind_raw[:].rearrange("p (t s) -> p t s", t=8)
            nc.sync.dma_start(
                ind_v[:, 0:4, :],
                ind_pos.ap()[:, g].rearrange("k w (hh s) -> (w hh) k s", hh=2))
            nc.sync.dma_start(
                ind_v[:, 4:8, :],
                ind_neg.ap()[:, g].rearrange("k w (hh s) -> (w hh) k s", hh=2))
            # load values [128, (t8, sig32)] f32
            val_raw = inp_pool.tile([P, 256], f32, name="val_raw")
            val_v = val_raw[:].rearrange("p (t s) -> p t s", t=8)
            nc.sync.dma_start(
                val_v[:, 0:4, :],
                val_pos.ap()[:, g].rearrange("k w (hh s) -> (w hh) k s", hh=2))
            nc.sync.dma_start(
                val_v[:, 4:8, :],
                val_neg.ap()[:, g].rearrange("k w (hh s) -> (w hh) k s", hh=2))

            # collision kill: kill[t] = 1 if any t' > t has equal index
            kill = prep_pool.tile([P, 256], i32, name="kill")
            nc.vector.memset(kill[:], 0)
            for d in range(1, 8):
                n = (8 - d) * 32
                eq_d = prep_pool.tile([P, 224], i32, name="eq_d")
                nc.vector.tensor_tensor(out=eq_d[:, 0:n], in0=ind_raw[:, 0:n],
                                        in1=ind_raw[:, 32 * d:256],
                                        op=Alu.is_equal)
                nc.vector.tensor_tensor(out=kill[:, 0:n], in0=kill[:, 0:n],
                                        in1=eq_d[:, 0:n], op=Alu.max)

            # m = ind + 128*(sig%4) + 64*h - 8192*kill
            u_t = prep_pool.tile([P, 256], i32, name="u_t")
            nc.vector.scalar_tensor_tensor(out=u_t[:], in0=ind_raw[:],
                                           scalar=1.0, in1=iota_cs[h][:],
                                           op0=Alu.mult, op1=Alu.add)
            m_t = prep_pool.tile([P, 256], i32, name="m_t")
            nc.vector.scalar_tensor_tensor(out=m_t[:], in0=kill[:],
                                           scalar=-8192.0, in1=u_t[:],
                                           op0=Alu.mult, op1=Alu.add)

            # reorder (t, q, s) -> (q, t, s); convert idx->i16, val->f16
            iv = idxs_p[:].rearrange("p (q g t s) -> p q g t s",
                                     q=8, g=2, t=8)[:, :, h]
            nc.vector.tensor_copy(
                iv, m_t[:].rearrange("p (t q s) -> p q t s", t=8, q=8))
            dv = data_p[:].rearrange("p (q g t s) -> p q g t s",
                                     q=8, g=2, t=8)[:, :, h]
            nc.scalar.copy(
                out=dv, in_=val_raw[:].rearrange("p (t q s) -> p q t s",
                                                 t=8, q=8))

        # GPSIMD scatter (both groups interleaved per op)
        for q in range(8):
            nc.gpsimd.local_scatter(
                xT[:, 512 * q:512 * (q + 1)],
                data_p[:, 64 * q:64 * (q + 1)],
                idxs_p[:, 64 * q:64 * (q + 1)],
                channels=P, num_elems=512, num_idxs=64)

        # transposes: per sigma, in [128, (g2, c64)] -> out [128=(g2,c64), p128]
        # x_sb free layout: (p128, sig32) so 512-contiguous chunks are
        # (p-group-of-16 x all sigma) = one staging buffer's spatial range.
        x_sb = xsb_pool.tile([P, 4096], f16, name=f"x_sb{pr}", tag="x")
        xTv = xT[:].rearrange("p (s gc) -> p s gc", s=32)
        x_v = x_sb[:].rearrange("c (pp s) -> c s pp", s=32)
        for sg in range(8):
            px = psx_pool.tile([P, 512], f16, name="px")
            for sl in range(4):
                sig = 4 * sg + sl
                nc.tensor.transpose(out=px[:, 128 * sl:128 * (sl + 1)],
                                    in_=xTv[:, sig], identity=ident[:])
            nc.scalar.copy(out=x_v[:, 4 * sg:4 * sg + 4, :],
                           in_=px[:].rearrange("c (sl pp) -> c sl pp", sl=4))
        x_tiles.append(x_sb)

    # ---- matmuls + staging + output DMA ----
    # staging buffer u covers a-range [8u, 8u+8) <=> p-range [16u, 16u+16)
    # buffer layout per partition (g,co): (a8, i4, b2, sig32, j4) = 8192 f32,
    # which is exactly DRAM element order -> fully contiguous 32KB/partition.
    y_flat = y.ap().rearrange("c wo ho -> c (wo ho)")
    copy_tick = 0
    for u in range(8):
        stg = stage_pool.tile([P, 8192], f32, name="stg", tag="stg")
        for ij in range(16):
            i, j = ij // 4, ij % 4
            py = psy_pool.tile([P, 512], f32, name="py")
            for pr in range(2):
                lhsT = (w_tiles[pr][:]
                        .rearrange("c (m i j) -> c m i j", m=64, i=4)
                        [:, :, i, j])
                rhs = x_tiles[pr][:, 512 * u:512 * (u + 1)]
                nc.tensor.matmul(out=py[64 * pr:64 * (pr + 1), :],
                                 lhsT=lhsT, rhs=rhs,
                                 start=True, stop=True,
                                 tile_position=(0, 64 * pr))
            # psum free = (p'16 = (a'8, b2), sig32) -> stg (a', i, b, sig, j)
            src = py[:].rearrange("c (a b s) -> c a b s", a=8, b=2)
            dst = (stg[:].rearrange("c (a i b s j) -> c i j a b s",
                                    a=8, i=4, b=2, s=32)[:, i, j])
            if copy_tick % 2 == 0:
                nc.vector.tensor_copy(dst, src)
            else:
                nc.scalar.copy(out=dst, in_=src)
            copy_tick += 1
        nc.sync.dma_start(y_flat[:, 8192 * u:8192 * (u + 1)], stg[:])


def _in_maps(inp):
    ind_pos, ind_neg = inp["ind_pos"], inp["ind_neg"]
    val_pos, val_neg = inp["val_pos"], inp["val_neg"]
    weight = inp["weight"]
    in_maps = []
    for c in range(N_CORES):
        b, hf = c // 2, c % 2
        gsl = slice(4 * hf, 4 * hf + 4)
        in_maps.append({
            "ind_pos": np.ascontiguousarray(ind_pos[b, :, gsl]),
            "ind_neg": np.ascontiguousarray(ind_neg[b, :, gsl]),
            "val_pos": np.ascontiguousarray(val_pos[b, :, gsl]),
            "val_neg": np.ascontiguousarray(val_neg[b, :, gsl]),
            "weight": np.ascontiguousarray(weight[256 * hf:256 * hf + 256]),
        })
    return in_maps


def _assemble(res):
    out = np.empty((4, 256, 256, 256), np.float32)
    for c in range(N_CORES):
        b, hf = c // 2, c % 2
        out[b, 128 * hf:128 * hf + 128] = res.results[c]["y"]
    return out


def kernel(ind_pos, ind_neg, val_pos, val_neg, weight):
    from concourse.bass_utils import run_bass_kernel_spmd
    nc = _build()
    in_maps = _in_maps(dict(ind_pos=np.asarray(ind_pos), ind_neg=np.asarray(ind_neg),
                            val_pos=np.asarray(val_pos), val_neg=np.asarray(val_neg),
                            weight=np.asarray(weight)))
    res = run_bass_kernel_spmd(nc, in_maps, core_ids=list(range(N_CORES)))
    return _assemble(res)
